# revision 25
# baseline (speedup 1.0000x reference)
"""Bass/Trainium2 kernel for nn_MultiHeadedAttention (GQA + RoPE + causal attention).

Sharding: 8 cores = 2 batch groups x 4 head-groups.
Core c: batch b=c//4, head group j=c%4 (q heads 4j..4j+3, kv head j).
Output projection is column-sharded after AllGathers of ctx^T within each
batch group; the host concatenates the disjoint output slices.

v3: single fused region. Projection rounds (output-major, 1 PSUM bank per
chain), attention blocks in (h,qb)-wavefront order, and output-projection
passes are emitted interleaved so TensorE stays busy while the Activation
engine runs exp and the collectives stream. exp runs on [128,1024] k-tile
pairs (two PSUM banks) to halve ACT-queue instruction+semaphore overhead;
softmax denominators are ones-matmuls per k-tile straight off the exp
output (no vector/gpsimd reduction on the critical path). Block-end
normalization and gathers are deferred into the next block's first pair
to avoid serializing the tensor queue. All inputs host-relaid so every
DMA has 2-16KB contiguous lines; output is bf16.
"""

import sys

sys.path.insert(0, "/opt/trn_rl_repo")
import numpy as np


B, S, HID = 2, 2048, 2048
NH, NKV, D = 16, 4, 128
N_CORES = 8
GROUPS = [[0, 1, 2, 3], [4, 5, 6, 7]]
HLOC = 4          # q heads per core
TB = 512          # token block
NTB = S // TB     # 4
HT = HID // 128   # 16 hid tiles
SCALE = float(D) ** -0.5

# wavefront order of attention blocks (head, qblock), by antidiagonal
WAVE = [(0, 0), (1, 0), (0, 1), (2, 0), (1, 1), (0, 2), (3, 0), (2, 1),
        (1, 2), (0, 3), (3, 1), (2, 2), (1, 3), (2, 3), (3, 2), (3, 3)]
# gathers fired after these blocks: (head, kind)
# kind: 0 = hf0 (qb0+qb1), 1 = hf1 (qb2+qb3), 2 = qb2 only, 3 = qb3 only
GATHER_AFTER = {
    (0, 1): (0, 0), (1, 1): (1, 0), (2, 1): (2, 0), (0, 3): (0, 1),
    (3, 1): (3, 0), (1, 3): (1, 1), (2, 3): (2, 1), (3, 2): (3, 2),
    (3, 3): (3, 3),
}

LAST_RESULTS = None  # stash for test harness timing


def _analyze_mask(mask):
    """Per (qblock, ktile-pair): live kt list and mixed pair-masks (deduped).

    Returns live (kt lists per qb), mixd2 {(qb, pair_idx): uniq2 id or None},
    uniq2 (list of [128, 1024] f32 pair-mask tiles, kT-major layout)."""
    maskb = np.asarray(mask).astype(bool)
    live = []
    tileinfo = {}
    for qb in range(NTB):
        lv = []
        for kt in range(S // 128):
            sub = maskb[qb * TB:(qb + 1) * TB, kt * 128:(kt + 1) * 128]
            if not sub.any():
                continue
            lv.append(kt)
            tileinfo[(qb, kt)] = None if sub.all() else \
                np.ascontiguousarray(sub.T.astype(np.float32))
        if len(lv) % 2:
            lv = lv[:-1] + [lv[-1], lv[-1]]  # shouldn't happen for causal
        live.append(lv)
    mixd2 = {}
    uniq2 = []
    lsmap = {}
    keys = {}
    onehalf = np.ones((128, TB), np.float32)
    for qb in range(NTB):
        lv = live[qb]
        for pi in range(len(lv) // 2):
            a, b_ = tileinfo[(qb, lv[2 * pi])], tileinfo[(qb, lv[2 * pi + 1])]
            ls0 = 0 if a is None else int(np.argmax(a.any(axis=0)))
            ls1 = 0 if b_ is None else int(np.argmax(b_.any(axis=0)))
            lsmap[(qb, pi)] = (ls0, ls1)
            if a is None and b_ is None:
                mixd2[(qb, pi)] = None
                continue
            comb = np.concatenate(
                [a if a is not None else onehalf,
                 b_ if b_ is not None else onehalf], axis=1)
            kb = comb.tobytes()
            if kb not in keys:
                keys[kb] = len(uniq2)
                uniq2.append(comb)
            mixd2[(qb, pi)] = keys[kb]
    return live, mixd2, uniq2, lsmap


def _build_program(live, mixd2, n_u2, lsmap):
    import concourse.bass as bass  # noqa: F401
    import concourse.mybir as mybir
    from concourse import bacc, tile

    f32 = mybir.dt.float32
    bf16 = mybir.dt.bfloat16
    EXP = mybir.ActivationFunctionType.Exp

    nc = bacc.Bacc("TRN2", target_bir_lowering=False, debug=False,
                   num_devices=N_CORES)

    # DRAM inputs, all pre-laid out host-side for contiguous partition lines.
    xt_d = nc.dram_tensor("xt_d", [128, NTB * HT * TB], bf16, kind="ExternalInput")
    wq_d = nc.dram_tensor("wq_d", [128, HLOC * HT * 128], bf16, kind="ExternalInput")
    wk_d = nc.dram_tensor("wk_d", [128, HT * 128], bf16, kind="ExternalInput")
    wv_d = nc.dram_tensor("wv_d", [128, HT * 128], bf16, kind="ExternalInput")
    wo_d = nc.dram_tensor("wo_d", [128, HT * TB], bf16, kind="ExternalInput")
    cosE = nc.dram_tensor("cosE", [D, S], bf16, kind="ExternalInput")
    sinP = nc.dram_tensor("sinP", [D, S], bf16, kind="ExternalInput")
    pswap = nc.dram_tensor("pswap", [128, 128], bf16, kind="ExternalInput")
    ident = nc.dram_tensor("ident", [128, 128], bf16, kind="ExternalInput")
    ones_in = nc.dram_tensor("ones_in", [128, 1], bf16, kind="ExternalInput")
    onesk1_in = nc.dram_tensor("onesk1_in", [1, 128], bf16, kind="ExternalInput")
    mmask2 = nc.dram_tensor("mmask2", [max(n_u2, 1) * 128, 2 * TB], bf16,
                            kind="ExternalInput")
    out_o = nc.dram_tensor("o", [128, HT * TB], bf16, kind="ExternalOutput")

    mm = nc.tensor.matmul

    with tile.TileContext(nc, num_cores=N_CORES) as tc:
        stk0 = nc.allow_low_precision("bf16 kernel; fp32 PSUM accumulate")
        stk0.__enter__()
        with (
            tc.tile_pool(name="const", bufs=1) as cpool,
            tc.tile_pool(name="wts", bufs=1) as wpool,
            tc.tile_pool(name="xin", bufs=2) as xin,
            tc.tile_pool(name="acts", bufs=1) as apool,
            tc.tile_pool(name="rst", bufs=2) as rst,
            tc.tile_pool(name="ex", bufs=4) as epool,
            tc.tile_pool(name="msc", bufs=2) as msc,
            tc.tile_pool(name="gsp", bufs=36) as gsp,
            tc.tile_pool(name="dram", bufs=1, space="DRAM") as dram,
            tc.tile_pool(name="pj", bufs=2, space="PSUM") as pj,
            tc.tile_pool(name="sp", bufs=2, space="PSUM") as sp,
            tc.tile_pool(name="cp", bufs=1, space="PSUM") as cp,
            tc.tile_pool(name="db", bufs=1, space="PSUM") as db,
        ):
            # ---------------- startup DMAs (scalar/Activation hwdge queue;
            # no exp has been emitted yet so nothing stalls behind them) ----
            ones_s = cpool.tile([128, 1], bf16, tag="ones")
            nc.scalar.dma_start(out=ones_s[:], in_=ones_in[:])
            # dummy exp: pull the ACT table load off the critical path
            scr = cpool.tile([128, 1], bf16, tag="scr")
            nc.scalar.activation(scr[:], ones_s[:], EXP, scale=1.0)

            xt_t = [xin.tile([128, HT * TB], bf16, tag="xt", name=f"xt{t}")
                    for t in range(NTB)]
            wq_s = wpool.tile([128, HLOC * HT * 128], bf16, tag="wq")
            nc.scalar.dma_start(out=wq_s[:, 0:HT * 128],
                                in_=wq_d[:, 0:HT * 128])
            for pc in range(4):
                nc.scalar.dma_start(out=xt_t[0][:, pc * 4 * TB:(pc + 1) * 4 * TB],
                                    in_=xt_d[:, pc * 4 * TB:(pc + 1) * 4 * TB])
            ps_s = wpool.tile([128, 128], bf16, tag="ps")
            nc.scalar.dma_start(out=ps_s[:], in_=pswap[:])
            cos_s = wpool.tile([D, S], bf16, tag="cos")
            nc.scalar.dma_start(out=cos_s[:], in_=cosE[:])
            sin_s = wpool.tile([D, S], bf16, tag="sin")
            nc.scalar.dma_start(out=sin_s[:], in_=sinP[:])
            for i in range(1, HLOC):
                nc.scalar.dma_start(out=wq_s[:, i * HT * 128:(i + 1) * HT * 128],
                                    in_=wq_d[:, i * HT * 128:(i + 1) * HT * 128])
            wk_s = wpool.tile([128, HT * 128], bf16, tag="wk")
            nc.scalar.dma_start(out=wk_s[:], in_=wk_d[:])
            wv_s = wpool.tile([128, HT * 128], bf16, tag="wv")
            nc.scalar.dma_start(out=wv_s[:], in_=wv_d[:])
            id_s = wpool.tile([128, 128], bf16, tag="id")
            nc.scalar.dma_start(out=id_s[:], in_=ident[:])
            onesk1 = cpool.tile([1, 128], bf16, tag="onesk1")
            nc.scalar.dma_start(out=onesk1[:], in_=onesk1_in[:])
            mm2_s = None
            if n_u2:
                mm2_s = cpool.tile([128, n_u2 * 2 * TB], bf16, tag="mm")
                nc.scalar.dma_start(
                    out=mm2_s[:].rearrange("p (u n) -> p u n", n=2 * TB),
                    in_=mmask2[:].rearrange("(u p) n -> p u n", p=128),
                )
            nc.scalar.dma_start(out=xt_t[1][:], in_=xt_d[:, HT * TB:2 * HT * TB])
            wo_s = wpool.tile([128, HT * TB], bf16, tag="wo")
            nc.scalar.dma_start(out=wo_s[:], in_=wo_d[:])

            # SBUF activation tensors
            qT_s = apool.tile([128, HLOC * S], bf16, tag="qT")
            kT_s = apool.tile([128, S], bf16, tag="kT")
            v_s = apool.tile([128, S], bf16, tag="v")
            ctxT_s = apool.tile([128, HLOC * S], bf16, tag="ctxT")

            # DRAM bounce/gather buffers: kind 0/1 are 1024-token halves,
            # kind 2/3 are 512-token quarters (head 3 only).
            HS = S // 2
            bounce = {}
            gath = {}
            for h in range(HLOC):
                for kind in ((0, 1) if h < 3 else (0, 2, 3)):
                    w = HS if kind < 2 else TB
                    bounce[(h, kind)] = dram.tile(
                        [128, w], bf16, tag=f"bn{h}{kind}", name=f"bn{h}{kind}")
                    gath[(h, kind)] = dram.tile(
                        [HLOC * 128, w], bf16, tag=f"g{h}{kind}",
                        name=f"g{h}{kind}")

            # ------------- projection chain emission (output-major) -------------
            # chain kinds: 0..3 = q heads, 4 = k, 5 = v
            def proj_chain_items(t, kind):
                """One closure per instruction of this chain."""
                items = []
                state = {}

                def alloc():
                    state['pps'] = pj.tile([128, TB], f32, tag="pps", name="pps")
                items.append(alloc)

                if kind <= 3:
                    w, off = wq_s, kind * HT * 128
                elif kind == 4:
                    w, off = wk_s, 0
                else:
                    w, off = wv_s, 0

                for h in range(HT):
                    def go_mm(h=h, w=w, off=off):
                        mm(state['pps'][:], w[:, off + h * 128: off + (h + 1) * 128],
                           xt_t[t][:, h * TB:(h + 1) * TB],
                           start=(h == 0), stop=(h == HT - 1))
                    items.append(go_mm)

                if kind <= 4:
                    if kind <= 3:
                        def dest():
                            return qT_s[:, kind * S + t * TB: kind * S + (t + 1) * TB]
                    else:
                        def dest():
                            return kT_s[:, t * TB:(t + 1) * TB]

                    def go_cast():
                        nc.vector.tensor_copy(dest(), state['pps'][:])
                    items.append(go_cast)

                    def go_sw():
                        state['sw'] = pj.tile([128, TB], f32, tag="pps", name="swp")
                        mm(state['sw'][:], ps_s[:], dest(), start=True, stop=True)
                    items.append(go_sw)

                    def go_m1():
                        state['swm'] = rst.tile([128, TB], bf16, tag="swm", name="swm")
                        nc.vector.tensor_mul(state['swm'][:], state['sw'][:],
                                             sin_s[:, t * TB:(t + 1) * TB])
                    items.append(go_m1)

                    def go_m2():
                        nc.gpsimd.tensor_mul(dest(), dest(),
                                             cos_s[:, t * TB:(t + 1) * TB])
                    items.append(go_m2)

                    def go_m3():
                        nc.gpsimd.tensor_add(dest(), dest(), state['swm'][:])
                    items.append(go_m3)
                else:
                    def go_vstg():
                        state['vstg'] = msc.tile([128, TB], bf16, tag="vstg",
                                                 name="vstg")
                        nc.vector.tensor_copy(state['vstg'][:], state['pps'][:])
                    items.append(go_vstg)

                    def go_tp_alloc():
                        state['tp'] = pj.tile([128, TB], bf16, tag="pps", name="tp")
                    items.append(go_tp_alloc)
                    for i in range(4):
                        def go_tr(i=i):
                            nc.tensor.transpose(
                                state['tp'][:, i * 128:(i + 1) * 128],
                                state['vstg'][:, i * 128:(i + 1) * 128], id_s[:])
                        items.append(go_tr)

                    def go_vcast():
                        nc.vector.tensor_copy(v_s[:, t * TB:(t + 1) * TB],
                                              state['tp'][:])
                    items.append(go_vcast)
                return items

            rounds_done = [False] * NTB

            def emit_round_now(t):
                for kind in range(6):
                    for it in proj_chain_items(t, kind):
                        it()
                rounds_done[t] = True

            def proj_gen():
                # rounds 1..3 lazily; xt DMAs for t=2,3 on the sync queue
                nc.sync.dma_start(out=xt_t[2][:],
                                  in_=xt_d[:, 2 * HT * TB:3 * HT * TB])
                for t in range(1, NTB):
                    if t == 3:
                        nc.sync.dma_start(
                            out=xt_t[3][:],
                            in_=xt_d[:, 3 * HT * TB:4 * HT * TB])
                    for kind in range(6):
                        for it in proj_chain_items(t, kind):
                            it()
                            yield t
                    rounds_done[t] = True
                while True:
                    yield None

            # ------------- phase 3: output projection passes -------------
            GIJ = [(h, j) for h in range(3) for j in range(HLOC)] + \
                  [(3, j) for j in range(HLOC)]

            gst = {}

            def emit_gs(h, j, q4):
                g = gsp.tile([128, TB], bf16, tag="gs", name="gs")
                if h < 3 or q4 < 2:
                    src = gath[(h, q4 // 2)][j * 128:(j + 1) * 128,
                                             (q4 % 2) * TB:(q4 % 2 + 1) * TB]
                else:
                    src = gath[(h, q4)][j * 128:(j + 1) * 128, :]
                nc.sync.dma_start(out=g[:], in_=src)
                gst[(h, j, q4)] = g

            def p3a_gen():
                # gs loads that depend only on the early (hf0) gathers of
                # heads 0-2: safe to drain well before phase 3 runs
                for q4 in range(2):
                    for (h, j) in GIJ:
                        if h == 3:
                            continue
                        emit_gs(h, j, q4)
                        yield 'gs'
                while True:
                    yield None

            def p3_gen():
                for q4 in range(4):
                    for (h, j) in GIJ:
                        if q4 < 2 and h < 3:
                            continue  # loaded by p3a_gen
                        emit_gs(h, j, q4)
                        yield 'gs'
                    for pa in range(q4 * 4, q4 * 4 + 4):
                        st = {}

                        def go_alloc():
                            st['ops'] = pj.tile([128, TB], f32, tag="pps",
                                                name="ops")
                        go_alloc()
                        yield 'a'
                        loc = pa % 4
                        for gi, (h, j) in enumerate(GIJ):
                            def go_mm(h=h, j=j, gi=gi, loc=loc, q4=q4):
                                g = 4 * j + h
                                mm(st['ops'][:],
                                   gst[(h, j, q4)][:, loc * 128:(loc + 1) * 128],
                                   wo_s[:, g * TB:(g + 1) * TB],
                                   start=(gi == 0), stop=(gi == HT - 1))
                            go_mm()
                            yield 'mm'

                        def go_out(pa=pa):
                            osb = msc.tile([128, TB], bf16, tag="osb", name="osb")
                            nc.vector.tensor_copy(osb[:], st['ops'][:])
                            nc.sync.dma_start(out=out_o[:, pa * TB:(pa + 1) * TB],
                                              in_=osb[:])
                        go_out()
                        yield 'o'
                while True:
                    yield None

            pgen = proj_gen()
            p3ag = p3a_gen()
            p3g = p3_gen()

            def drain(gen, n):
                for _ in range(n):
                    next(gen)

            def emit_gather(h, kind):
                if kind < 2:
                    src = ctxT_s[:, h * S + kind * HS: h * S + (kind + 1) * HS]
                else:
                    src = ctxT_s[:, h * S + kind * TB: h * S + (kind + 1) * TB]
                bn, gt = bounce[(h, kind)], gath[(h, kind)]
                nc.sync.dma_start(out=bn[:], in_=src)
                nc.gpsimd.collective_compute(
                    "AllGather",
                    mybir.AluOpType.bypass,
                    replica_groups=GROUPS,
                    ins=[bn.opt()],
                    outs=[gt.opt()],
                )

            # ------------- attention block emission -------------
            def emit_block(h, qb, carry, pdrain=4, p3gen=None, p3drain=0):
                """carry: deferred (finisher, gather) from the previous block,
                emitted after this block's first pair of score matmuls.

                Score/ctx matmuls are column-narrowed to each k-tile's live
                q range (causal intra-tile sparsity). The softmax denominator
                is: gpsimd half-sum per pair (off the critical path, emitted
                with one-pair lag), then a ones-matmul per pair on TensorE."""
                lv = live[qb]
                npair = len(lv) // 2
                cps = cp.tile([128, TB], f32, tag="cps", name="cps")
                dps = db.tile([1, TB], f32, tag="dps", name="dps")
                qslice = qT_s[:, h * S + qb * TB: h * S + (qb + 1) * TB]
                pend = []  # lagged denominator ones-matmuls

                def flush_pend():
                    exs_, pi_, ls_ = pend.pop(0)
                    mm(dps[:, ls_:TB], ones_s[:], exs_[:, ls_:TB],
                       start=(pi_ == 0), stop=(pi_ == npair - 1))

                for pi in range(npair):
                    k0, k1 = lv[2 * pi], lv[2 * pi + 1]
                    ls0, ls1 = lsmap[(qb, pi)]
                    first, last = (pi == 0), (pi == npair - 1)
                    sps2 = sp.tile([128, 2 * TB], f32, tag="sps", name="sps")
                    # half0 narrowed to its live columns; half1 full so the
                    # contiguous exp read [ls0:2TB] only sees written PSUM
                    mm(sps2[:, ls0:TB], kT_s[:, k0 * 128:(k0 + 1) * 128],
                       qslice[:, ls0:TB], start=True, stop=True)
                    mm(sps2[:, TB:2 * TB], kT_s[:, k1 * 128:(k1 + 1) * 128],
                       qslice, start=True, stop=True)
                    if first and carry:
                        fin, gargs = carry
                        fin()
                        if gargs:
                            emit_gather(*gargs)
                    ex2 = epool.tile([128, 2 * TB], bf16, tag="ex", name="ex")
                    nc.scalar.activation(ex2[:, ls0:2 * TB], sps2[:, ls0:2 * TB],
                                         EXP, scale=SCALE)
                    u2 = mixd2[(qb, pi)]
                    if u2 is not None:
                        # zeroes the below-diagonal and stale/garbage columns
                        nc.vector.tensor_mul(
                            ex2[:, ls0:2 * TB], ex2[:, ls0:2 * TB],
                            mm2_s[:, u2 * 2 * TB + ls0:(u2 + 1) * 2 * TB])
                    mm(cps[:, ls0:TB], v_s[:, k0 * 128:(k0 + 1) * 128],
                       ex2[:, ls0:TB], start=first, stop=False)
                    mm(cps[:, ls1:TB], v_s[:, k1 * 128:(k1 + 1) * 128],
                       ex2[:, TB + ls1:2 * TB], start=False, stop=last)
                    exs = epool.tile([128, TB], bf16, tag="exs", name="exs",
                                     bufs=3)
                    nc.vector.tensor_add(exs[:, ls0:TB], ex2[:, ls0:TB],
                                         ex2[:, TB + ls0:2 * TB])
                    pend.append((exs, pi, ls0))
                    if len(pend) > 2:
                        flush_pend()
                    drain(pgen, pdrain)
                    if p3drain:
                        drain(p3gen, p3drain)

                def finish():
                    while pend:
                        flush_pend()
                    rc = msc.tile([1, TB], f32, tag="rc", name="rc")
                    nc.vector.reciprocal_approx_fast(rc[:], dps[:])
                    rcb = msc.tile([1, TB], bf16, tag="rcb", name="rcb")
                    nc.vector.tensor_copy(rcb[:], rc[:])
                    bps = sp.tile([128, TB], f32, tag="sps", name="bps")
                    mm(bps[:], onesk1[:], rcb[:], start=True, stop=True)
                    bcs = msc.tile([128, TB], bf16, tag="bcs", name="bcs")
                    nc.vector.tensor_copy(bcs[:], bps[:])
                    nc.vector.tensor_mul(
                        ctxT_s[:, h * S + qb * TB: h * S + (qb + 1) * TB],
                        cps[:], bcs[:])
                return finish

            # ------------- main emission -------------
            emit_round_now(0)

            carry = None
            for (h, qb) in WAVE:
                while not rounds_done[qb]:
                    drain(pgen, 8)
                if (h, qb) in ((1, 3), (2, 3)):
                    gen2, r2 = p3ag, 2
                elif (h, qb) == (3, 2):
                    gen2, r2 = p3g, 12
                else:
                    gen2, r2 = None, 0
                fin = emit_block(h, qb, carry, pdrain=4, p3gen=gen2,
                                 p3drain=r2)
                g = GATHER_AFTER.get((h, qb))
                carry = (fin, g)
            # flush the last block's finisher + gather
            fin, g = carry
            fin()
            if g:
                emit_gather(*g)

            # drain remaining phase-3 work
            while True:
                try:
                    if next(p3g) is None:
                        break
                except StopIteration:
                    break
        stk0.__exit__(None, None, None)
    nc.compile()
    return nc


def kernel(x, wq, wk, wv, wo, cos, sin, mask):
    global LAST_RESULTS
    import ml_dtypes
    from concourse.bass_utils import run_bass_kernel_spmd

    bfnp = ml_dtypes.bfloat16
    x = np.asarray(x, np.float32)
    wq = np.asarray(wq, np.float32)
    wk = np.asarray(wk, np.float32)
    wv = np.asarray(wv, np.float32)
    wo = np.asarray(wo, np.float32)
    cos = np.asarray(cos, np.float32)
    sin = np.asarray(sin, np.float32)

    live, mixd2, uniq2, lsmap = _analyze_mask(mask)
    n_u2 = len(uniq2)
    mmask2 = (np.concatenate(uniq2, axis=0) if n_u2
              else np.zeros((128, 2 * TB), np.float32))

    cosE = np.repeat(cos, 2, axis=1).T
    sp = np.repeat(sin, 2, axis=1).copy()
    sp[:, 0::2] *= -1.0
    sinP = sp.T
    pswap = np.zeros((128, 128), np.float32)
    pswap[np.arange(128), np.arange(128) ^ 1] = 1.0
    ident = np.eye(128, dtype=np.float32)

    nc = _build_program(live, mixd2, n_u2, lsmap)

    def b(a):
        return np.ascontiguousarray(np.asarray(a).astype(bfnp))

    in_maps = []
    for c in range(N_CORES):
        bb, j = c // 4, c % 4
        # x tiles: [p, t*8192 + h*512 + c] = x[bb, t*512+c, h*128+p]
        xT = x[bb].T  # [hid, tok]
        xt_l = (xT.reshape(HT, 128, NTB, TB).transpose(1, 2, 0, 3)
                .reshape(128, NTB * HT * TB))
        # wq chain-major: [p, i*2048 + h*128 + d] = wq[h*128+p, i*128+d]
        wqc = wq[:, 512 * j:512 * (j + 1)]
        wq_l = (wqc.reshape(HT, 128, HLOC, 128).transpose(1, 2, 0, 3)
                .reshape(128, HLOC * HT * 128))
        wkc = wk[:, 128 * j:128 * (j + 1)]
        wk_l = wkc.reshape(HT, 128, 128).transpose(1, 0, 2).reshape(128, HT * 128)
        wvc = wv[:, 128 * j:128 * (j + 1)]
        wv_l = wvc.reshape(HT, 128, 128).transpose(1, 0, 2).reshape(128, HT * 128)
        # wo g-major: [p, g*512 + c] = wo[g*128+p, c]
        woc = wo[:, 512 * j:512 * (j + 1)]
        wo_l = woc.reshape(HT, 128, TB).transpose(1, 0, 2).reshape(128, HT * TB)
        in_maps.append({
            "xt_d": b(xt_l),
            "wq_d": b(wq_l),
            "wk_d": b(wk_l),
            "wv_d": b(wv_l),
            "wo_d": b(wo_l),
            "cosE": b(cosE), "sinP": b(sinP), "pswap": b(pswap),
            "ident": b(ident),
            "ones_in": b(np.ones((128, 1), np.float32)),
            "onesk1_in": b(np.ones((1, 128), np.float32)),
            "mmask2": b(mmask2),
        })

    res = run_bass_kernel_spmd(nc, in_maps, list(range(N_CORES)))
    LAST_RESULTS = res

    out = np.empty((B, S, HID), np.float32)
    for c in range(N_CORES):
        bb, j = c // 4, c % 4
        o = np.asarray(res.results[c]["o"], np.float32)  # [128, 16*512]
        out[bb, :, 512 * j:512 * (j + 1)] = (
            o.reshape(128, HT, TB).transpose(1, 0, 2).reshape(S, TB))
    return out
